# revision 14
# baseline (speedup 1.0000x reference)
"""Trainium2 Bass kernel for nn_Discriminator (segment_reduce, 8 cores).

Math (collapsed form of the reference):
  The reference projects the full embedding table (emb = E @ W_i.T + b_i),
  gathers pos/neg rows, does a segment-mean over pos rows, and scores each
  row with a bilinear form against its segment embedding.  Everything is
  linear, so it collapses to operations on RAW embedding rows:

    msum[s]  = sum of raw E rows of segment s's pos samples         [256]
    grid[s]  = (W_i/128) msum[s] + b_i
    h[s]     = Wb grid[s]                  (Wb = W_k[0])
    u[s]     = W_i^T h[s];   c[s] = b_i . h[s] + b_k
    logit[n] = E[idx[n]] . u[seg(n)] + c[seg(n)]

Sharding: data-parallel over samples, segments kept whole per core
(core k owns segments [k*128, (k+1)*128)).  Fully local, no collectives.

Device strategy (v4):
  The host pre-gathers each core's sample rows from the embedding table,
  casts them to fp16 (2e-2 tolerance; fp16 keeps logit error ~3e-3), and
  lays them out TRANSPOSED (feature-on-partition, two 128-feature halves)
  in the exact SBUF tile layout.  The device then:
    - streams the rows with large linear DMAs spread over FOUR DMA queues
      (sync, scalar, gpsimd, vector) so the transfers run concurrently;
      the schedule below keeps every queue ~48us busy and feeds tiles in
      consumption order,
    - computes per-group segment sums with a fp16 halving tree (DVE 2x
      mode) plus one final free-axis reduce,
    - runs the tiny 256x256 chain per group of 16 segments on PE (fp16
      weights packed into a single const DMA, f32 PSUM accumulation),
    - computes every per-row dot product as a 1-column PE matmul
      (lhsT = transposed row tile, rhs = u halves in fp16), accumulating
      both feature halves plus a ones-row matmul that seeds the PSUM
      column block with c[seg],
    - DMAs each group's [128, 96] PSUM logit block straight to DRAM.
  Roofline: 50MB/core of sample rows over 4 concurrent DMA queues.
"""

import numpy as np

import concourse.bass as bass
import concourse.bacc as bacc
import concourse.mybir as mybir
from concourse import bass_utils
from concourse.tile import TileContext

F32 = mybir.dt.float32
F16 = mybir.dt.float16

N_NODES = 200000
H = 256
N_SEG = 1024
SEG_SZ = 128          # rows per segment (asserted at runtime)
N_POS = N_SEG * SEG_SZ          # 131072
NEG_RATIO = 5
N_NEG = N_POS * NEG_RATIO       # 655360
N_CORES = 8

SEG_PC = N_SEG // N_CORES       # 128 segments per core
POS_PC = N_POS // N_CORES       # 16384
NEG_PC = N_NEG // N_CORES       # 81920
P = 128
POS_BLK = POS_PC // P           # 128 blocks (block == segment for pos)
NEG_BLK = NEG_PC // P           # 640 blocks (5 consecutive per segment)
TOT_BLK = POS_BLK + NEG_BLK     # 768 logit columns

GSEG = 16                       # segments per group
NGRP = SEG_PC // GSEG           # 8 groups
GBLK = GSEG * NEG_RATIO         # 80 neg blocks per group
PCOL = GSEG * 2 * P             # 4096 fp16 cols per pos group
HCOL = GSEG * P                 # 2048 cols per feature half (pos group)
NCOL = GBLK * 2 * P             # 20480 fp16 cols per neg group tile
NHC = GBLK * P                  # 10240 cols per feature half (neg group)
NEG_BUFS = 3
PPT = 4                         # pos groups per streamed pos tile
GCOL = 96                       # psum logit columns per group (16 pos + 80 neg)

# packed fp16 weights: w_iT/128 | wbT | w_ext ([W_i | b_i])
W1 = H + 1
OFF_WI = 0
OFF_WB = 2 * H
OFF_WE = 4 * H
WPACK = 4 * H + 2 * W1          # 1538 cols

# neg stream plan: per group, ordered list of (queue, blk_lo, blk_hi, tail)
# tail=True segments are emitted after the group's chain (DVE self-load).
NEG_PLAN = [
    [("act", 0, 80, False)],
    [("pool", 0, 80, False)],
    [("act", 0, 80, False)],
    [("pool", 0, 80, False)],
    [("sync", 0, 40, False), ("sync", 40, 80, False)],
    [("act", 0, 80, False)],
    [("pool", 0, 80, False)],
    [("sync", 0, 40, False), ("act", 40, 60, False), ("pool", 60, 80, False)],
]
OUT_QUEUE = ["act", "pool", "act", "pool", "pool", "act", "pool", "act"]

_CACHED = None


def _build_module() -> bass.Bass:
    # Bacc (not raw Bass): its compile() pass splits multi-sem waits into
    # event semaphores — walrus rejects >1 sync wait per instruction.
    nc = bacc.Bacc("TRN2", target_bir_lowering=False, debug=False)

    wpack_d = nc.dram_tensor("wpack", [P, WPACK], F16, kind="ExternalInput")
    bpack_d = nc.dram_tensor("bpack", [P, 3], F32, kind="ExternalInput")
    posT_d = nc.dram_tensor("posT", [P, NGRP * PCOL], F16, kind="ExternalInput")
    negT_d = nc.dram_tensor("negT", [P, NGRP * NCOL], F16, kind="ExternalInput")
    logits_d = nc.dram_tensor("logits", [P, NGRP * GCOL], F16,
                              kind="ExternalOutput")

    def q(name):
        return {"sync": nc.sync, "act": nc.scalar, "pool": nc.gpsimd,
                "vector": nc.vector}[name]

    with TileContext(nc) as tc:
        with (
            tc.tile_pool(name="const", bufs=1) as const,
            tc.tile_pool(name="pospool", bufs=2) as pospool,
            tc.tile_pool(name="negpool", bufs=NEG_BUFS) as negpool,
            tc.tile_pool(name="grp", bufs=2) as grp,
            tc.tile_pool(name="chain", bufs=2, space="PSUM") as chainp,
            tc.tile_pool(name="lg", bufs=4, space="PSUM") as lgp,
        ):
            # ---- pos stream + constants on the sync queue ----
            ones16 = const.tile([1, P], F16, tag="ones16")
            nc.gpsimd.memset(ones16[:], 1.0)

            pos_tiles = []
            wp = None
            for i in range(2):
                pt = pospool.tile([P, PPT * PCOL], F16, tag="pos")
                for h in range(2):
                    nc.sync.dma_start(
                        pt[:, h * 2 * PCOL:(h + 1) * 2 * PCOL],
                        posT_d[:, (i * PPT + h * 2) * PCOL:
                               (i * PPT + (h + 1) * 2) * PCOL])
                pos_tiles.append(pt)
                if i == 0:
                    wp = const.tile([P, WPACK], F16, tag="wpack")
                    nc.sync.dma_start(wp[:], wpack_d[:, :])
                    bp = const.tile([P, 3], F32, tag="bpack")
                    nc.sync.dma_start(bp[:], bpack_d[:, :])

            neg_tiles = [None] * NGRP

            def emit_neg_seg(g, queue, lo, hi):
                nt = neg_tiles[g]
                sb = nt[:].rearrange("p (j c) -> p j c", j=2)
                dr = negT_d[:, g * NCOL:(g + 1) * NCOL].rearrange(
                    "p (j c) -> p j c", j=2)
                q(queue).dma_start(sb[:, :, lo * P:hi * P],
                                   dr[:, :, lo * P:hi * P])

            plg_tiles = [None] * NGRP
            logits_sb = const.tile([P, NGRP * GCOL], F16, tag="logits")

            def emit_lg_copy(g, queue):
                # drain group g's PSUM block to SBUF, freeing its bank
                dst = logits_sb[:, g * GCOL:(g + 1) * GCOL]
                if queue == "act":
                    nc.scalar.copy(out=dst, in_=plg_tiles[g][:])
                else:
                    q(queue).tensor_copy(dst, plg_tiles[g][:])

            # ---- per group of GSEG segments: sums + chain + dots ----
            for g in range(NGRP):
                nt_new = negpool.tile([P, NCOL], F16, tag="neg")
                neg_tiles[g] = nt_new
                if g >= 2:
                    emit_lg_copy(g - 2, "act" if g % 2 == 0 else "pool")
                for queue, lo, hi, tail in NEG_PLAN[g]:
                    if not tail:
                        emit_neg_seg(g, queue, lo, hi)

                pg_ap = pos_tiles[g // PPT][:, (g % PPT) * PCOL:
                                            (g % PPT + 1) * PCOL]
                pv = pg_ap.rearrange("p (j s r) -> p j s r", j=2, s=GSEG)

                # segment sums via fp16 halving tree (DVE 2x) + final reduce
                s1 = grp.tile([P, 2048], F16, tag="s1")
                s1v = s1[:].rearrange("p (j s r) -> p j s r", j=2, s=GSEG)
                nc.vector.tensor_tensor(
                    out=s1v, in0=pv[:, :, :, 0:64], in1=pv[:, :, :, 64:128],
                    op=mybir.AluOpType.add)
                s2 = grp.tile([P, 1024], F16, tag="s2")
                s2v = s2[:].rearrange("p (j s r) -> p j s r", j=2, s=GSEG)
                nc.vector.tensor_tensor(
                    out=s2v, in0=s1v[:, :, :, 0:32], in1=s1v[:, :, :, 32:64],
                    op=mybir.AluOpType.add)
                s3 = grp.tile([P, 512], F16, tag="s3")
                s3v = s3[:].rearrange("p (j s r) -> p j s r", j=2, s=GSEG)
                nc.vector.tensor_tensor(
                    out=s3v, in0=s2v[:, :, :, 0:16], in1=s2v[:, :, :, 16:32],
                    op=mybir.AluOpType.add)
                mT16 = grp.tile([P, 2 * GSEG], F16, tag="mT16")
                with nc.allow_low_precision(reason="fp16 tail of segment sum"):
                    nc.vector.tensor_reduce(
                        out=mT16[:], in_=s3v,
                        op=mybir.AluOpType.add, axis=mybir.AxisListType.X)

                # G_T = (W_i/128) @ Msum_T + b_i
                gT = grp.tile([P, 2 * GSEG], F16, tag="gT")
                for t in range(2):
                    pg = chainp.tile([P, GSEG], F32, tag="chain")
                    for j in range(2):
                        nc.tensor.matmul(
                            out=pg[:],
                            lhsT=wp[:, OFF_WI + j * H + t * P:
                                    OFF_WI + j * H + t * P + P],
                            rhs=mT16[:, j * GSEG:(j + 1) * GSEG],
                            start=(j == 0),
                            stop=(j == 1),
                        )
                    nc.vector.tensor_scalar(
                        out=gT[:, t * GSEG:(t + 1) * GSEG], in0=pg[:],
                        scalar1=bp[:, t:t + 1], scalar2=None,
                        op0=mybir.AluOpType.add,
                    )

                # H_T = Wb @ G_T
                hT = grp.tile([P, 2 * GSEG], F16, tag="hT")
                for t in range(2):
                    ph = chainp.tile([P, GSEG], F32, tag="chain")
                    for j in range(2):
                        nc.tensor.matmul(
                            out=ph[:],
                            lhsT=wp[:, OFF_WB + j * H + t * P:
                                    OFF_WB + j * H + t * P + P],
                            rhs=gT[:, j * GSEG:(j + 1) * GSEG],
                            start=(j == 0),
                            stop=(j == 1),
                        )
                    nc.vector.tensor_copy(hT[:, t * GSEG:(t + 1) * GSEG], ph[:])

                # U_T halves (fp16 for the dot matmuls): u16[p, t*16+s]
                u16 = grp.tile([P, 2 * GSEG], F16, tag="u16")
                for t in range(2):
                    pu = chainp.tile([P, GSEG], F32, tag="chain")
                    for j in range(2):
                        nc.tensor.matmul(
                            out=pu[:],
                            lhsT=wp[:, OFF_WE + j * W1 + t * P:
                                    OFF_WE + j * W1 + t * P + P],
                            rhs=hT[:, j * GSEG:(j + 1) * GSEG],
                            start=(j == 0),
                            stop=(j == 1),
                        )
                    nc.vector.tensor_copy(u16[:, t * GSEG:(t + 1) * GSEG], pu[:])

                # c row: b_i . h + b_k, replicated into the 96-col layout
                puc = chainp.tile([1, GSEG], F32, tag="chain")
                for j in range(2):
                    nc.tensor.matmul(
                        out=puc[:],
                        lhsT=wp[:, OFF_WE + j * W1 + H: OFF_WE + j * W1 + H + 1],
                        rhs=hT[:, j * GSEG:(j + 1) * GSEG],
                        start=(j == 0),
                        stop=(j == 1),
                    )
                uc16 = grp.tile([1, GSEG], F16, tag="uc16")
                nc.vector.tensor_scalar(
                    out=uc16[:], in0=puc[:], scalar1=bp[0:1, 2:3],
                    scalar2=None, op0=mybir.AluOpType.add,
                )
                c6 = grp.tile([1, GSEG * 6], F16, tag="c6")
                nc.vector.tensor_copy(c6[:1, 0:GSEG], uc16[:1, :])
                for r in range(NEG_RATIO):
                    nc.vector.tensor_copy(
                        c6[:1, GSEG + r:GSEG + r + 5 * (GSEG - 1) + 1:5],
                        uc16[:1, :])

                # late (tail) stream segments owned by this group
                for queue, lo, hi, tail in NEG_PLAN[g]:
                    if tail:
                        emit_neg_seg(g, queue, lo, hi)

                # dots: psum cols [0:16) pos, [16:96) neg; seeded with c
                plg_t = lgp.tile([P, GCOL], F32, tag="lg")
                plg_tiles[g] = plg_t
                plg = plg_t[:]
                nc.tensor.matmul(
                    out=plg, lhsT=ones16[:], rhs=c6[:1, :],
                    start=True, stop=False, skip_group_check=True,
                )
                for sl in range(GSEG):
                    for j in range(2):
                        nc.tensor.matmul(
                            out=plg[:, sl:sl + 1],
                            lhsT=pg_ap[:, j * HCOL + sl * P:
                                       j * HCOL + sl * P + P],
                            rhs=u16[:, j * GSEG + sl:j * GSEG + sl + 1],
                            start=False,
                            stop=(j == 1),
                            skip_group_check=True,
                        )
                nt = neg_tiles[g]
                last = (len(NEG_PLAN[g]) - 1, NEG_PLAN[g][-1][2] - 1)
                for si, (queue, lo, hi, tail) in enumerate(NEG_PLAN[g]):
                    for b in range(lo, hi):
                        sl = b // NEG_RATIO
                        for j in range(2):
                            nc.tensor.matmul(
                                out=plg[:, GSEG + b:GSEG + b + 1],
                                lhsT=nt[:, j * NHC + b * P:
                                        j * NHC + b * P + P],
                                rhs=u16[:, j * GSEG + sl:j * GSEG + sl + 1],
                                start=False,
                                stop=((si, b) == last),
                                skip_group_check=True,
                            )

            # ---- drain the last two groups, then one DMA out ----
            emit_lg_copy(NGRP - 2, "vector")
            emit_lg_copy(NGRP - 1, "vector")
            nc.sync.dma_start(logits_d[:, :], logits_sb[:])

    nc.compile()
    return nc


def get_module() -> bass.Bass:
    global _CACHED
    if _CACHED is None:
        _CACHED = _build_module()
    return _CACHED


def make_in_maps(inputs: dict) -> list[dict]:
    emb16 = np.asarray(inputs["embedding"], dtype=np.float32).astype(np.float16)
    gs = np.asarray(inputs["grid_sizes"]).astype(np.int64)
    pos_s = np.asarray(inputs["pos_samples"]).astype(np.int64)
    neg_s = np.asarray(inputs["neg_samples"]).astype(np.int64)
    W_i = np.asarray(inputs["W_i"], dtype=np.float32)
    b_i = np.asarray(inputs["b_i"], dtype=np.float32)
    Wb = np.asarray(inputs["W_k"], dtype=np.float32)[0]
    b_kv = np.asarray(inputs["b_k"], dtype=np.float32)

    if not (gs.shape == (N_SEG,) and np.all(gs == SEG_SZ)):
        raise RuntimeError("kernel assumes grid_sizes == 128 everywhere")
    assert pos_s.shape == (N_POS,) and neg_s.shape == (N_NEG,)

    # packed fp16 weights, all as lhsT tiles [p, j, cols]
    w_iT_t = (W_i / float(SEG_SZ)).T.reshape(2, P, H).transpose(1, 0, 2)
    wbT_t = Wb.T.reshape(2, P, H).transpose(1, 0, 2)
    W_ext = np.concatenate([W_i, b_i[:, None]], axis=1)        # [256, 257]
    w_ext_t = W_ext.reshape(2, P, W1).transpose(1, 0, 2)
    wpack_np = np.concatenate(
        [w_iT_t.reshape(P, 2 * H), wbT_t.reshape(P, 2 * H),
         w_ext_t.reshape(P, 2 * W1)], axis=1).astype(np.float16)
    bpack_np = np.concatenate(
        [np.ascontiguousarray(b_i.reshape(2, P).T),
         np.full((P, 1), b_kv[0], np.float32)], axis=1)

    in_maps = []
    for k in range(N_CORES):
        pos_rows = emb16[pos_s[k * POS_PC:(k + 1) * POS_PC]]   # [16384, 256]
        neg_rows = emb16[neg_s[k * NEG_PC:(k + 1) * NEG_PC]]   # [81920, 256]
        # (g, s, r, j, p) -> [p, g, j, s, r]
        posT_np = np.ascontiguousarray(
            pos_rows.reshape(NGRP, GSEG, P, 2, P).transpose(4, 0, 3, 1, 2)
        ).reshape(P, NGRP * PCOL)
        # (g, b, r, j, p) -> [p, g, j, b, r]
        negT_np = np.ascontiguousarray(
            neg_rows.reshape(NGRP, GBLK, P, 2, P).transpose(4, 0, 3, 1, 2)
        ).reshape(P, NGRP * NCOL)
        in_maps.append({
            "wpack": wpack_np,
            "bpack": bpack_np,
            "posT": posT_np,
            "negT": negT_np,
        })
    return in_maps


def assemble_output(core_outs: list[np.ndarray]) -> np.ndarray:
    pos_parts, neg_parts = [], []
    for k in range(N_CORES):
        o = np.asarray(core_outs[k]).astype(np.float32).reshape(P, NGRP, GCOL)
        # pos block b = g*16+sl lives at o[:, g, sl]; neg block q = g*80+lq
        # at o[:, g, 16+lq]; output is block-major then row.
        pos_parts.append(o[:, :, :GSEG].transpose(1, 2, 0).ravel())
        neg_parts.append(o[:, :, GSEG:].transpose(1, 2, 0).ravel())
    return np.concatenate(pos_parts + neg_parts).astype(np.float32)


def kernel(**inputs) -> np.ndarray:
    nc = get_module()
    in_maps = make_in_maps(inputs)
    res = bass_utils.run_bass_kernel_spmd(
        nc, in_maps, core_ids=list(range(N_CORES)))
    return assemble_output([r["logits"] for r in res.results])


# revision 15
# speedup vs baseline: 1.0026x; 1.0026x over previous
"""Trainium2 Bass kernel for nn_Discriminator (segment_reduce, 8 cores).

Math (collapsed form of the reference):
  The reference projects the full embedding table (emb = E @ W_i.T + b_i),
  gathers pos/neg rows, does a segment-mean over pos rows, and scores each
  row with a bilinear form against its segment embedding.  Everything is
  linear, so it collapses to operations on RAW embedding rows:

    msum[s]  = sum of raw E rows of segment s's pos samples         [256]
    grid[s]  = (W_i/128) msum[s] + b_i
    h[s]     = Wb grid[s]                  (Wb = W_k[0])
    u[s]     = W_i^T h[s];   c[s] = b_i . h[s] + b_k
    logit[n] = E[idx[n]] . u[seg(n)] + c[seg(n)]

Sharding: data-parallel over samples, segments kept whole per core
(core k owns segments [k*128, (k+1)*128)).  Fully local, no collectives.

Device strategy (v4):
  The host pre-gathers each core's sample rows from the embedding table,
  casts them to fp16 (2e-2 tolerance; fp16 keeps logit error ~3e-3), and
  lays them out TRANSPOSED (feature-on-partition, two 128-feature halves)
  in the exact SBUF tile layout.  The device then:
    - streams the rows with large linear DMAs spread over FOUR DMA queues
      (sync, scalar, gpsimd, vector) so the transfers run concurrently;
      the schedule below keeps every queue ~48us busy and feeds tiles in
      consumption order,
    - computes per-group segment sums with a fp16 halving tree (DVE 2x
      mode) plus one final free-axis reduce,
    - runs the tiny 256x256 chain per group of 16 segments on PE (fp16
      weights packed into a single const DMA, f32 PSUM accumulation),
    - computes every per-row dot product as a 1-column PE matmul
      (lhsT = transposed row tile, rhs = u halves in fp16), accumulating
      both feature halves plus a ones-row matmul that seeds the PSUM
      column block with c[seg],
    - DMAs each group's [128, 96] PSUM logit block straight to DRAM.
  Roofline: 50MB/core of sample rows over 4 concurrent DMA queues.
"""

import numpy as np

import concourse.bass as bass
import concourse.bacc as bacc
import concourse.mybir as mybir
from concourse import bass_utils
from concourse.tile import TileContext

F32 = mybir.dt.float32
F16 = mybir.dt.float16

N_NODES = 200000
H = 256
N_SEG = 1024
SEG_SZ = 128          # rows per segment (asserted at runtime)
N_POS = N_SEG * SEG_SZ          # 131072
NEG_RATIO = 5
N_NEG = N_POS * NEG_RATIO       # 655360
N_CORES = 8

SEG_PC = N_SEG // N_CORES       # 128 segments per core
POS_PC = N_POS // N_CORES       # 16384
NEG_PC = N_NEG // N_CORES       # 81920
P = 128
POS_BLK = POS_PC // P           # 128 blocks (block == segment for pos)
NEG_BLK = NEG_PC // P           # 640 blocks (5 consecutive per segment)
TOT_BLK = POS_BLK + NEG_BLK     # 768 logit columns

GSEG = 16                       # segments per group
NGRP = SEG_PC // GSEG           # 8 groups
GBLK = GSEG * NEG_RATIO         # 80 neg blocks per group
PCOL = GSEG * 2 * P             # 4096 fp16 cols per pos group
HCOL = GSEG * P                 # 2048 cols per feature half (pos group)
NCOL = GBLK * 2 * P             # 20480 fp16 cols per neg group tile
NHC = GBLK * P                  # 10240 cols per feature half (neg group)
NEG_BUFS = 3
PPT = 4                         # pos groups per streamed pos tile
GCOL = 96                       # psum logit columns per group (16 pos + 80 neg)

# packed fp16 weights: w_iT/128 | wbT | w_ext ([W_i | b_i])
W1 = H + 1
OFF_WI = 0
OFF_WB = 2 * H
OFF_WE = 4 * H
WPACK = 4 * H + 2 * W1          # 1538 cols

# neg stream plan: per group, ordered list of (queue, blk_lo, blk_hi, tail)
# tail=True segments are emitted after the group's chain (DVE self-load).
NEG_PLAN = [
    [("act", 0, 80, False)],
    [("pool", 0, 80, False)],
    [("act", 0, 80, False)],
    [("pool", 0, 80, False)],
    [("sync", 0, 40, False), ("sync", 40, 80, False)],
    [("act", 0, 80, False)],
    [("pool", 0, 80, False)],
    [("sync", 0, 40, False), ("act", 40, 60, False), ("pool", 60, 80, False)],
]
OUT_QUEUE = ["act", "pool", "act", "pool", "pool", "act", "pool", "act"]

_CACHED = None


def _build_module() -> bass.Bass:
    # Bacc (not raw Bass): its compile() pass splits multi-sem waits into
    # event semaphores — walrus rejects >1 sync wait per instruction.
    nc = bacc.Bacc("TRN2", target_bir_lowering=False, debug=False)

    wpack_d = nc.dram_tensor("wpack", [P, WPACK], F16, kind="ExternalInput")
    bpack_d = nc.dram_tensor("bpack", [P, 3], F32, kind="ExternalInput")
    posT_d = nc.dram_tensor("posT", [P, NGRP * PCOL], F16, kind="ExternalInput")
    negT_d = nc.dram_tensor("negT", [P, NGRP * NCOL], F16, kind="ExternalInput")
    logits_d = nc.dram_tensor("logits", [P, NGRP * GCOL], F16,
                              kind="ExternalOutput")

    def q(name):
        return {"sync": nc.sync, "act": nc.scalar, "pool": nc.gpsimd,
                "vector": nc.vector}[name]

    with TileContext(nc) as tc:
        with (
            tc.tile_pool(name="const", bufs=1) as const,
            tc.tile_pool(name="pospool", bufs=2) as pospool,
            tc.tile_pool(name="negpool", bufs=NEG_BUFS) as negpool,
            tc.tile_pool(name="grp", bufs=2) as grp,
            tc.tile_pool(name="chain", bufs=2, space="PSUM") as chainp,
            tc.tile_pool(name="lg", bufs=4, space="PSUM") as lgp,
        ):
            # ---- pos stream + constants on the sync queue ----
            ones16 = const.tile([1, P], F16, tag="ones16")
            nc.gpsimd.memset(ones16[:], 1.0)

            pos_tiles = []
            wp = None
            for i in range(2):
                pt = pospool.tile([P, PPT * PCOL], F16, tag="pos")
                for h in range(2):
                    nc.sync.dma_start(
                        pt[:, h * 2 * PCOL:(h + 1) * 2 * PCOL],
                        posT_d[:, (i * PPT + h * 2) * PCOL:
                               (i * PPT + (h + 1) * 2) * PCOL])
                pos_tiles.append(pt)
                if i == 0:
                    wp = const.tile([P, WPACK], F16, tag="wpack")
                    nc.sync.dma_start(wp[:], wpack_d[:, :])
                    bp = const.tile([P, 3], F32, tag="bpack")
                    nc.sync.dma_start(bp[:], bpack_d[:, :])

            neg_tiles = [None] * NGRP

            def emit_neg_seg(g, queue, lo, hi):
                nt = neg_tiles[g]
                sb = nt[:].rearrange("p (j c) -> p j c", j=2)
                dr = negT_d[:, g * NCOL:(g + 1) * NCOL].rearrange(
                    "p (j c) -> p j c", j=2)
                q(queue).dma_start(sb[:, :, lo * P:hi * P],
                                   dr[:, :, lo * P:hi * P])

            plg_tiles = [None] * NGRP
            logits_sb = const.tile([P, NGRP * GCOL], F16, tag="logits")

            def emit_lg_copy(g, queue):
                # drain group g's PSUM block to SBUF, freeing its bank
                dst = logits_sb[:, g * GCOL:(g + 1) * GCOL]
                if queue == "act":
                    nc.scalar.copy(out=dst, in_=plg_tiles[g][:])
                else:
                    q(queue).tensor_copy(dst, plg_tiles[g][:])

            # ---- per group of GSEG segments: sums + chain + dots ----
            for g in range(NGRP):
                nt_new = negpool.tile([P, NCOL], F16, tag="neg")
                neg_tiles[g] = nt_new
                if g >= 2:
                    emit_lg_copy(g - 2, "act")
                for queue, lo, hi, tail in NEG_PLAN[g]:
                    if not tail:
                        emit_neg_seg(g, queue, lo, hi)

                pg_ap = pos_tiles[g // PPT][:, (g % PPT) * PCOL:
                                            (g % PPT + 1) * PCOL]
                pv = pg_ap.rearrange("p (j s r) -> p j s r", j=2, s=GSEG)

                # segment sums via fp16 halving tree (DVE 2x) + final reduce
                s1 = grp.tile([P, 2048], F16, tag="s1")
                s1v = s1[:].rearrange("p (j s r) -> p j s r", j=2, s=GSEG)
                nc.vector.tensor_tensor(
                    out=s1v, in0=pv[:, :, :, 0:64], in1=pv[:, :, :, 64:128],
                    op=mybir.AluOpType.add)
                s2 = grp.tile([P, 1024], F16, tag="s2")
                s2v = s2[:].rearrange("p (j s r) -> p j s r", j=2, s=GSEG)
                nc.vector.tensor_tensor(
                    out=s2v, in0=s1v[:, :, :, 0:32], in1=s1v[:, :, :, 32:64],
                    op=mybir.AluOpType.add)
                s3 = grp.tile([P, 512], F16, tag="s3")
                s3v = s3[:].rearrange("p (j s r) -> p j s r", j=2, s=GSEG)
                nc.vector.tensor_tensor(
                    out=s3v, in0=s2v[:, :, :, 0:16], in1=s2v[:, :, :, 16:32],
                    op=mybir.AluOpType.add)
                mT16 = grp.tile([P, 2 * GSEG], F16, tag="mT16")
                with nc.allow_low_precision(reason="fp16 tail of segment sum"):
                    nc.vector.tensor_reduce(
                        out=mT16[:], in_=s3v,
                        op=mybir.AluOpType.add, axis=mybir.AxisListType.X)

                # G_T = (W_i/128) @ Msum_T + b_i
                gT = grp.tile([P, 2 * GSEG], F16, tag="gT")
                for t in range(2):
                    pg = chainp.tile([P, GSEG], F32, tag="chain")
                    for j in range(2):
                        nc.tensor.matmul(
                            out=pg[:],
                            lhsT=wp[:, OFF_WI + j * H + t * P:
                                    OFF_WI + j * H + t * P + P],
                            rhs=mT16[:, j * GSEG:(j + 1) * GSEG],
                            start=(j == 0),
                            stop=(j == 1),
                        )
                    nc.vector.tensor_scalar(
                        out=gT[:, t * GSEG:(t + 1) * GSEG], in0=pg[:],
                        scalar1=bp[:, t:t + 1], scalar2=None,
                        op0=mybir.AluOpType.add,
                    )

                # H_T = Wb @ G_T
                hT = grp.tile([P, 2 * GSEG], F16, tag="hT")
                for t in range(2):
                    ph = chainp.tile([P, GSEG], F32, tag="chain")
                    for j in range(2):
                        nc.tensor.matmul(
                            out=ph[:],
                            lhsT=wp[:, OFF_WB + j * H + t * P:
                                    OFF_WB + j * H + t * P + P],
                            rhs=gT[:, j * GSEG:(j + 1) * GSEG],
                            start=(j == 0),
                            stop=(j == 1),
                        )
                    nc.vector.tensor_copy(hT[:, t * GSEG:(t + 1) * GSEG], ph[:])

                # U_T halves (fp16 for the dot matmuls): u16[p, t*16+s]
                u16 = grp.tile([P, 2 * GSEG], F16, tag="u16")
                for t in range(2):
                    pu = chainp.tile([P, GSEG], F32, tag="chain")
                    for j in range(2):
                        nc.tensor.matmul(
                            out=pu[:],
                            lhsT=wp[:, OFF_WE + j * W1 + t * P:
                                    OFF_WE + j * W1 + t * P + P],
                            rhs=hT[:, j * GSEG:(j + 1) * GSEG],
                            start=(j == 0),
                            stop=(j == 1),
                        )
                    nc.vector.tensor_copy(u16[:, t * GSEG:(t + 1) * GSEG], pu[:])

                # c row: b_i . h + b_k, replicated into the 96-col layout
                puc = chainp.tile([1, GSEG], F32, tag="chain")
                for j in range(2):
                    nc.tensor.matmul(
                        out=puc[:],
                        lhsT=wp[:, OFF_WE + j * W1 + H: OFF_WE + j * W1 + H + 1],
                        rhs=hT[:, j * GSEG:(j + 1) * GSEG],
                        start=(j == 0),
                        stop=(j == 1),
                    )
                uc16 = grp.tile([1, GSEG], F16, tag="uc16")
                nc.vector.tensor_scalar(
                    out=uc16[:], in0=puc[:], scalar1=bp[0:1, 2:3],
                    scalar2=None, op0=mybir.AluOpType.add,
                )
                c6 = grp.tile([1, GSEG * 6], F16, tag="c6")
                nc.vector.tensor_copy(c6[:1, 0:GSEG], uc16[:1, :])
                for r in range(NEG_RATIO):
                    nc.vector.tensor_copy(
                        c6[:1, GSEG + r:GSEG + r + 5 * (GSEG - 1) + 1:5],
                        uc16[:1, :])

                # late (tail) stream segments owned by this group
                for queue, lo, hi, tail in NEG_PLAN[g]:
                    if tail:
                        emit_neg_seg(g, queue, lo, hi)

                # dots: psum cols [0:16) pos, [16:96) neg; seeded with c
                plg_t = lgp.tile([P, GCOL], F32, tag="lg")
                plg_tiles[g] = plg_t
                plg = plg_t[:]
                nc.tensor.matmul(
                    out=plg, lhsT=ones16[:], rhs=c6[:1, :],
                    start=True, stop=False, skip_group_check=True,
                )
                for sl in range(GSEG):
                    for j in range(2):
                        nc.tensor.matmul(
                            out=plg[:, sl:sl + 1],
                            lhsT=pg_ap[:, j * HCOL + sl * P:
                                       j * HCOL + sl * P + P],
                            rhs=u16[:, j * GSEG + sl:j * GSEG + sl + 1],
                            start=False,
                            stop=(j == 1),
                            skip_group_check=True,
                        )
                nt = neg_tiles[g]
                last = (len(NEG_PLAN[g]) - 1, NEG_PLAN[g][-1][2] - 1)
                for si, (queue, lo, hi, tail) in enumerate(NEG_PLAN[g]):
                    for b in range(lo, hi):
                        sl = b // NEG_RATIO
                        for j in range(2):
                            nc.tensor.matmul(
                                out=plg[:, GSEG + b:GSEG + b + 1],
                                lhsT=nt[:, j * NHC + b * P:
                                        j * NHC + b * P + P],
                                rhs=u16[:, j * GSEG + sl:j * GSEG + sl + 1],
                                start=False,
                                stop=((si, b) == last),
                                skip_group_check=True,
                            )

            # ---- drain the last two groups, then one DMA out ----
            emit_lg_copy(NGRP - 2, "vector")
            emit_lg_copy(NGRP - 1, "vector")
            nc.sync.dma_start(logits_d[:, :], logits_sb[:])

    nc.compile()
    return nc


def get_module() -> bass.Bass:
    global _CACHED
    if _CACHED is None:
        _CACHED = _build_module()
    return _CACHED


def make_in_maps(inputs: dict) -> list[dict]:
    emb16 = np.asarray(inputs["embedding"], dtype=np.float32).astype(np.float16)
    gs = np.asarray(inputs["grid_sizes"]).astype(np.int64)
    pos_s = np.asarray(inputs["pos_samples"]).astype(np.int64)
    neg_s = np.asarray(inputs["neg_samples"]).astype(np.int64)
    W_i = np.asarray(inputs["W_i"], dtype=np.float32)
    b_i = np.asarray(inputs["b_i"], dtype=np.float32)
    Wb = np.asarray(inputs["W_k"], dtype=np.float32)[0]
    b_kv = np.asarray(inputs["b_k"], dtype=np.float32)

    if not (gs.shape == (N_SEG,) and np.all(gs == SEG_SZ)):
        raise RuntimeError("kernel assumes grid_sizes == 128 everywhere")
    assert pos_s.shape == (N_POS,) and neg_s.shape == (N_NEG,)

    # packed fp16 weights, all as lhsT tiles [p, j, cols]
    w_iT_t = (W_i / float(SEG_SZ)).T.reshape(2, P, H).transpose(1, 0, 2)
    wbT_t = Wb.T.reshape(2, P, H).transpose(1, 0, 2)
    W_ext = np.concatenate([W_i, b_i[:, None]], axis=1)        # [256, 257]
    w_ext_t = W_ext.reshape(2, P, W1).transpose(1, 0, 2)
    wpack_np = np.concatenate(
        [w_iT_t.reshape(P, 2 * H), wbT_t.reshape(P, 2 * H),
         w_ext_t.reshape(P, 2 * W1)], axis=1).astype(np.float16)
    bpack_np = np.concatenate(
        [np.ascontiguousarray(b_i.reshape(2, P).T),
         np.full((P, 1), b_kv[0], np.float32)], axis=1)

    in_maps = []
    for k in range(N_CORES):
        pos_rows = emb16[pos_s[k * POS_PC:(k + 1) * POS_PC]]   # [16384, 256]
        neg_rows = emb16[neg_s[k * NEG_PC:(k + 1) * NEG_PC]]   # [81920, 256]
        # (g, s, r, j, p) -> [p, g, j, s, r]
        posT_np = np.ascontiguousarray(
            pos_rows.reshape(NGRP, GSEG, P, 2, P).transpose(4, 0, 3, 1, 2)
        ).reshape(P, NGRP * PCOL)
        # (g, b, r, j, p) -> [p, g, j, b, r]
        negT_np = np.ascontiguousarray(
            neg_rows.reshape(NGRP, GBLK, P, 2, P).transpose(4, 0, 3, 1, 2)
        ).reshape(P, NGRP * NCOL)
        in_maps.append({
            "wpack": wpack_np,
            "bpack": bpack_np,
            "posT": posT_np,
            "negT": negT_np,
        })
    return in_maps


def assemble_output(core_outs: list[np.ndarray]) -> np.ndarray:
    pos_parts, neg_parts = [], []
    for k in range(N_CORES):
        o = np.asarray(core_outs[k]).astype(np.float32).reshape(P, NGRP, GCOL)
        # pos block b = g*16+sl lives at o[:, g, sl]; neg block q = g*80+lq
        # at o[:, g, 16+lq]; output is block-major then row.
        pos_parts.append(o[:, :, :GSEG].transpose(1, 2, 0).ravel())
        neg_parts.append(o[:, :, GSEG:].transpose(1, 2, 0).ravel())
    return np.concatenate(pos_parts + neg_parts).astype(np.float32)


def kernel(**inputs) -> np.ndarray:
    nc = get_module()
    in_maps = make_in_maps(inputs)
    res = bass_utils.run_bass_kernel_spmd(
        nc, in_maps, core_ids=list(range(N_CORES)))
    return assemble_output([r["logits"] for r in res.results])


# revision 16
# speedup vs baseline: 1.1180x; 1.1152x over previous
"""Trainium2 Bass kernel for nn_Discriminator (segment_reduce, 8 cores).

Math (collapsed form of the reference):
  The reference projects the full embedding table (emb = E @ W_i.T + b_i),
  gathers pos/neg rows, does a segment-mean over pos rows, and scores each
  row with a bilinear form against its segment embedding.  Everything is
  linear, so it collapses to operations on RAW embedding rows:

    msum[s]  = sum of raw E rows of segment s's pos samples         [256]
    grid[s]  = (W_i/128) msum[s] + b_i
    h[s]     = Wb grid[s]                  (Wb = W_k[0])
    u[s]     = W_i^T h[s];   c[s] = b_i . h[s] + b_k
    logit[n] = E[idx[n]] . u[seg(n)] + c[seg(n)]

Sharding: data-parallel over samples, segments kept whole per core
(core k owns segments [k*128, (k+1)*128)).  Fully local, no collectives.

Device strategy (v4):
  The host pre-gathers each core's sample rows from the embedding table,
  casts them to fp16 (2e-2 tolerance; fp16 keeps logit error ~3e-3), and
  lays them out TRANSPOSED (feature-on-partition, two 128-feature halves)
  in the exact SBUF tile layout.  The device then:
    - streams the rows with large linear DMAs spread over FOUR DMA queues
      (sync, scalar, gpsimd, vector) so the transfers run concurrently;
      the schedule below keeps every queue ~48us busy and feeds tiles in
      consumption order,
    - computes per-group segment sums with a fp16 halving tree (DVE 2x
      mode) plus one final free-axis reduce,
    - runs the tiny 256x256 chain per group of 16 segments on PE (fp16
      weights packed into a single const DMA, f32 PSUM accumulation),
    - computes every per-row dot product as a 1-column PE matmul
      (lhsT = transposed row tile, rhs = u halves in fp16), accumulating
      both feature halves plus a ones-row matmul that seeds the PSUM
      column block with c[seg],
    - DMAs each group's [128, 96] PSUM logit block straight to DRAM.
  Roofline: 50MB/core of sample rows over 4 concurrent DMA queues.
"""

import numpy as np

import concourse.bass as bass
import concourse.bacc as bacc
import concourse.mybir as mybir
from concourse import bass_utils
from concourse.tile import TileContext

F32 = mybir.dt.float32
F16 = mybir.dt.float16

N_NODES = 200000
H = 256
N_SEG = 1024
SEG_SZ = 128          # rows per segment (asserted at runtime)
N_POS = N_SEG * SEG_SZ          # 131072
NEG_RATIO = 5
N_NEG = N_POS * NEG_RATIO       # 655360
N_CORES = 8

SEG_PC = N_SEG // N_CORES       # 128 segments per core
POS_PC = N_POS // N_CORES       # 16384
NEG_PC = N_NEG // N_CORES       # 81920
P = 128
POS_BLK = POS_PC // P           # 128 blocks (block == segment for pos)
NEG_BLK = NEG_PC // P           # 640 blocks (5 consecutive per segment)
TOT_BLK = POS_BLK + NEG_BLK     # 768 logit columns

GSEG = 16                       # segments per group
NGRP = SEG_PC // GSEG           # 8 groups
GBLK = GSEG * NEG_RATIO         # 80 neg blocks per group
PCOL = GSEG * 2 * P             # 4096 fp16 cols per pos group
HCOL = GSEG * P                 # 2048 cols per feature half (pos group)
NCOL = GBLK * 2 * P             # 20480 fp16 cols per neg group tile
NHC = GBLK * P                  # 10240 cols per feature half (neg group)
NEG_BUFS = 6                    # half-group tiles in flight
HBLK = 40                       # neg blocks per half tile
HNC = HBLK * P                  # 5120 cols per feature half within a half tile
PPT = 4                         # pos groups per streamed pos tile
GCOL = 96                       # psum logit columns per group (16 pos + 80 neg)

# packed fp16 weights: w_iT/128 | wbT | w_ext ([W_i | b_i])
W1 = H + 1
OFF_WI = 0
OFF_WB = 2 * H
OFF_WE = 4 * H
WPACK = 4 * H + 2 * W1          # 1538 cols

# neg stream plan: per group, ordered list of (queue, blk_lo, blk_hi).
# Ranges never cross the half-tile boundary (block 40).  Order within a
# group = consumption (dot emission) order.
NEG_PLAN = [
    [("act", 0, 40), ("pool", 40, 80)],
    [("act", 0, 40), ("pool", 40, 80)],
    [("act", 0, 40), ("pool", 40, 80)],
    [("pool", 40, 80), ("sync", 0, 40)],
    [("act", 0, 40), ("sync", 40, 80)],
    [("act", 0, 40), ("pool", 40, 80)],
    [("act", 0, 40), ("pool", 40, 80)],
    [("sync", 0, 40), ("act", 40, 60), ("pool", 60, 80)],
]

_CACHED = None


def _build_module() -> bass.Bass:
    # Bacc (not raw Bass): its compile() pass splits multi-sem waits into
    # event semaphores — walrus rejects >1 sync wait per instruction.
    nc = bacc.Bacc("TRN2", target_bir_lowering=False, debug=False)

    wpack_d = nc.dram_tensor("wpack", [P, WPACK], F16, kind="ExternalInput")
    bpack_d = nc.dram_tensor("bpack", [P, 3], F32, kind="ExternalInput")
    posT_d = nc.dram_tensor("posT", [P, NGRP * PCOL], F16, kind="ExternalInput")
    negT_d = nc.dram_tensor("negT", [P, NGRP * NCOL], F16, kind="ExternalInput")
    logits_d = nc.dram_tensor("logits", [P, NGRP * GCOL], F16,
                              kind="ExternalOutput")

    def q(name):
        return {"sync": nc.sync, "act": nc.scalar, "pool": nc.gpsimd,
                "vector": nc.vector}[name]

    with TileContext(nc) as tc:
        with (
            tc.tile_pool(name="const", bufs=1) as const,
            tc.tile_pool(name="pospool", bufs=2) as pospool,
            tc.tile_pool(name="negpool", bufs=NEG_BUFS) as negpool,
            tc.tile_pool(name="grp", bufs=2) as grp,
            tc.tile_pool(name="chain", bufs=2, space="PSUM") as chainp,
            tc.tile_pool(name="lg", bufs=6, space="PSUM") as lgp,
        ):
            # ---- pos stream + constants on the sync queue ----
            ones16 = const.tile([1, P], F16, tag="ones16")
            nc.gpsimd.memset(ones16[:], 1.0)

            wp = const.tile([P, WPACK], F16, tag="wpack")
            nc.sync.dma_start(wp[:], wpack_d[:, :])
            bp = const.tile([P, 3], F32, tag="bpack")
            nc.sync.dma_start(bp[:], bpack_d[:, :])
            pos_tiles = []
            for i in range(2):
                pt = pospool.tile([P, PPT * PCOL], F16, tag="pos")
                for h in range(2):
                    nc.sync.dma_start(
                        pt[:, h * 2 * PCOL:(h + 1) * 2 * PCOL],
                        posT_d[:, (i * PPT + h * 2) * PCOL:
                               (i * PPT + (h + 1) * 2) * PCOL])
                pos_tiles.append(pt)

            neg_tiles = [None] * (2 * NGRP)   # half tiles: index 2g+h

            def emit_neg_seg(g, queue, lo, hi):
                h = lo // HBLK
                assert hi <= (h + 1) * HBLK
                nt = neg_tiles[2 * g + h]
                sb = nt[:].rearrange("p (j c) -> p j c", j=2)
                # dram half-tile: cols j*10240 + (b-40h)*128 within group g
                dr = negT_d[:, g * NCOL:(g + 1) * NCOL].rearrange(
                    "p (j c) -> p j c", j=2)
                q(queue).dma_start(
                    sb[:, :, (lo - h * HBLK) * P:(hi - h * HBLK) * P],
                    dr[:, :, lo * P:hi * P])

            plg_tiles = [None] * NGRP
            logits_sb = const.tile([P, NGRP * GCOL], F16, tag="logits")

            def emit_lg_copy(g, queue):
                # drain group g's PSUM block to SBUF, freeing its bank
                q(queue).tensor_copy(
                    logits_sb[:, g * GCOL:(g + 1) * GCOL], plg_tiles[g][:])

            # ---- per group of GSEG segments: sums + chain + dots ----
            for g in range(NGRP):
                nt_a = negpool.tile([P, 2 * HNC], F16, tag="neg")
                neg_tiles[2 * g] = nt_a
                nt_b = negpool.tile([P, 2 * HNC], F16, tag="neg")
                neg_tiles[2 * g + 1] = nt_b
                if g >= 4:
                    emit_lg_copy(g - 4, "vector")
                for queue, lo, hi in NEG_PLAN[g]:
                    emit_neg_seg(g, queue, lo, hi)

                pg_ap = pos_tiles[g // PPT][:, (g % PPT) * PCOL:
                                            (g % PPT + 1) * PCOL]
                pv = pg_ap.rearrange("p (j s r) -> p j s r", j=2, s=GSEG)

                # segment sums via fp16 halving tree (DVE 2x) + final reduce
                s1 = grp.tile([P, 2048], F16, tag="s1")
                s1v = s1[:].rearrange("p (j s r) -> p j s r", j=2, s=GSEG)
                nc.vector.tensor_tensor(
                    out=s1v, in0=pv[:, :, :, 0:64], in1=pv[:, :, :, 64:128],
                    op=mybir.AluOpType.add)
                s2 = grp.tile([P, 1024], F16, tag="s2")
                s2v = s2[:].rearrange("p (j s r) -> p j s r", j=2, s=GSEG)
                nc.vector.tensor_tensor(
                    out=s2v, in0=s1v[:, :, :, 0:32], in1=s1v[:, :, :, 32:64],
                    op=mybir.AluOpType.add)
                s3 = grp.tile([P, 512], F16, tag="s3")
                s3v = s3[:].rearrange("p (j s r) -> p j s r", j=2, s=GSEG)
                nc.vector.tensor_tensor(
                    out=s3v, in0=s2v[:, :, :, 0:16], in1=s2v[:, :, :, 16:32],
                    op=mybir.AluOpType.add)
                mT16 = grp.tile([P, 2 * GSEG], F16, tag="mT16")
                with nc.allow_low_precision(reason="fp16 tail of segment sum"):
                    nc.vector.tensor_reduce(
                        out=mT16[:], in_=s3v,
                        op=mybir.AluOpType.add, axis=mybir.AxisListType.X)

                # G_T = (W_i/128) @ Msum_T + b_i
                gT = grp.tile([P, 2 * GSEG], F16, tag="gT")
                for t in range(2):
                    pg = chainp.tile([P, GSEG], F32, tag="chain")
                    for j in range(2):
                        nc.tensor.matmul(
                            out=pg[:],
                            lhsT=wp[:, OFF_WI + j * H + t * P:
                                    OFF_WI + j * H + t * P + P],
                            rhs=mT16[:, j * GSEG:(j + 1) * GSEG],
                            start=(j == 0),
                            stop=(j == 1),
                        )
                    nc.vector.tensor_scalar(
                        out=gT[:, t * GSEG:(t + 1) * GSEG], in0=pg[:],
                        scalar1=bp[:, t:t + 1], scalar2=None,
                        op0=mybir.AluOpType.add,
                    )

                # H_T = Wb @ G_T
                hT = grp.tile([P, 2 * GSEG], F16, tag="hT")
                for t in range(2):
                    ph = chainp.tile([P, GSEG], F32, tag="chain")
                    for j in range(2):
                        nc.tensor.matmul(
                            out=ph[:],
                            lhsT=wp[:, OFF_WB + j * H + t * P:
                                    OFF_WB + j * H + t * P + P],
                            rhs=gT[:, j * GSEG:(j + 1) * GSEG],
                            start=(j == 0),
                            stop=(j == 1),
                        )
                    nc.vector.tensor_copy(hT[:, t * GSEG:(t + 1) * GSEG], ph[:])

                # U_T halves (fp16 for the dot matmuls): u16[p, t*16+s]
                u16 = grp.tile([P, 2 * GSEG], F16, tag="u16")
                for t in range(2):
                    pu = chainp.tile([P, GSEG], F32, tag="chain")
                    for j in range(2):
                        nc.tensor.matmul(
                            out=pu[:],
                            lhsT=wp[:, OFF_WE + j * W1 + t * P:
                                    OFF_WE + j * W1 + t * P + P],
                            rhs=hT[:, j * GSEG:(j + 1) * GSEG],
                            start=(j == 0),
                            stop=(j == 1),
                        )
                    nc.vector.tensor_copy(u16[:, t * GSEG:(t + 1) * GSEG], pu[:])

                # c row: b_i . h + b_k, replicated into the 96-col layout
                puc = chainp.tile([1, GSEG], F32, tag="chain")
                for j in range(2):
                    nc.tensor.matmul(
                        out=puc[:],
                        lhsT=wp[:, OFF_WE + j * W1 + H: OFF_WE + j * W1 + H + 1],
                        rhs=hT[:, j * GSEG:(j + 1) * GSEG],
                        start=(j == 0),
                        stop=(j == 1),
                    )
                uc16 = grp.tile([1, GSEG], F16, tag="uc16")
                nc.vector.tensor_scalar(
                    out=uc16[:], in0=puc[:], scalar1=bp[0:1, 2:3],
                    scalar2=None, op0=mybir.AluOpType.add,
                )
                c6 = grp.tile([1, GSEG * 6], F16, tag="c6")
                nc.vector.tensor_copy(c6[:1, 0:GSEG], uc16[:1, :])
                for r in range(NEG_RATIO):
                    nc.vector.tensor_copy(
                        c6[:1, GSEG + r:GSEG + r + 5 * (GSEG - 1) + 1:5],
                        uc16[:1, :])

                # dots: psum cols [0:16) pos, [16:96) neg; seeded with c
                plg_t = lgp.tile([P, GCOL], F32, tag="lg")
                plg_tiles[g] = plg_t
                plg = plg_t[:]
                nc.tensor.matmul(
                    out=plg, lhsT=ones16[:], rhs=c6[:1, :],
                    start=True, stop=False, skip_group_check=True,
                )
                for sl in range(GSEG):
                    for j in range(2):
                        nc.tensor.matmul(
                            out=plg[:, sl:sl + 1],
                            lhsT=pg_ap[:, j * HCOL + sl * P:
                                       j * HCOL + sl * P + P],
                            rhs=u16[:, j * GSEG + sl:j * GSEG + sl + 1],
                            start=False,
                            stop=(j == 1),
                            skip_group_check=True,
                        )
                last = (len(NEG_PLAN[g]) - 1, NEG_PLAN[g][-1][2] - 1)
                for si, (queue, lo, hi) in enumerate(NEG_PLAN[g]):
                    for b in range(lo, hi):
                        h = b // HBLK
                        nt = neg_tiles[2 * g + h]
                        bl = b - h * HBLK
                        sl = b // NEG_RATIO
                        for j in range(2):
                            nc.tensor.matmul(
                                out=plg[:, GSEG + b:GSEG + b + 1],
                                lhsT=nt[:, j * HNC + bl * P:
                                        j * HNC + bl * P + P],
                                rhs=u16[:, j * GSEG + sl:j * GSEG + sl + 1],
                                start=False,
                                stop=((si, b) == last),
                                skip_group_check=True,
                            )

            # ---- drain the last four groups, then one DMA out ----
            for g in range(NGRP - 4, NGRP):
                emit_lg_copy(g, "vector")
            nc.sync.dma_start(logits_d[:, :], logits_sb[:])

    nc.compile()
    return nc


def get_module() -> bass.Bass:
    global _CACHED
    if _CACHED is None:
        _CACHED = _build_module()
    return _CACHED


def make_in_maps(inputs: dict) -> list[dict]:
    emb16 = np.asarray(inputs["embedding"], dtype=np.float32).astype(np.float16)
    gs = np.asarray(inputs["grid_sizes"]).astype(np.int64)
    pos_s = np.asarray(inputs["pos_samples"]).astype(np.int64)
    neg_s = np.asarray(inputs["neg_samples"]).astype(np.int64)
    W_i = np.asarray(inputs["W_i"], dtype=np.float32)
    b_i = np.asarray(inputs["b_i"], dtype=np.float32)
    Wb = np.asarray(inputs["W_k"], dtype=np.float32)[0]
    b_kv = np.asarray(inputs["b_k"], dtype=np.float32)

    if not (gs.shape == (N_SEG,) and np.all(gs == SEG_SZ)):
        raise RuntimeError("kernel assumes grid_sizes == 128 everywhere")
    assert pos_s.shape == (N_POS,) and neg_s.shape == (N_NEG,)

    # packed fp16 weights, all as lhsT tiles [p, j, cols]
    w_iT_t = (W_i / float(SEG_SZ)).T.reshape(2, P, H).transpose(1, 0, 2)
    wbT_t = Wb.T.reshape(2, P, H).transpose(1, 0, 2)
    W_ext = np.concatenate([W_i, b_i[:, None]], axis=1)        # [256, 257]
    w_ext_t = W_ext.reshape(2, P, W1).transpose(1, 0, 2)
    wpack_np = np.concatenate(
        [w_iT_t.reshape(P, 2 * H), wbT_t.reshape(P, 2 * H),
         w_ext_t.reshape(P, 2 * W1)], axis=1).astype(np.float16)
    bpack_np = np.concatenate(
        [np.ascontiguousarray(b_i.reshape(2, P).T),
         np.full((P, 1), b_kv[0], np.float32)], axis=1)

    in_maps = []
    for k in range(N_CORES):
        pos_rows = emb16[pos_s[k * POS_PC:(k + 1) * POS_PC]]   # [16384, 256]
        neg_rows = emb16[neg_s[k * NEG_PC:(k + 1) * NEG_PC]]   # [81920, 256]
        # (g, s, r, j, p) -> [p, g, j, s, r]
        posT_np = np.ascontiguousarray(
            pos_rows.reshape(NGRP, GSEG, P, 2, P).transpose(4, 0, 3, 1, 2)
        ).reshape(P, NGRP * PCOL)
        # (g, b, r, j, p) -> [p, g, j, b, r]
        negT_np = np.ascontiguousarray(
            neg_rows.reshape(NGRP, GBLK, P, 2, P).transpose(4, 0, 3, 1, 2)
        ).reshape(P, NGRP * NCOL)
        in_maps.append({
            "wpack": wpack_np,
            "bpack": bpack_np,
            "posT": posT_np,
            "negT": negT_np,
        })
    return in_maps


def assemble_output(core_outs: list[np.ndarray]) -> np.ndarray:
    pos_parts, neg_parts = [], []
    for k in range(N_CORES):
        o = np.asarray(core_outs[k]).astype(np.float32).reshape(P, NGRP, GCOL)
        # pos block b = g*16+sl lives at o[:, g, sl]; neg block q = g*80+lq
        # at o[:, g, 16+lq]; output is block-major then row.
        pos_parts.append(o[:, :, :GSEG].transpose(1, 2, 0).ravel())
        neg_parts.append(o[:, :, GSEG:].transpose(1, 2, 0).ravel())
    return np.concatenate(pos_parts + neg_parts).astype(np.float32)


def kernel(**inputs) -> np.ndarray:
    nc = get_module()
    in_maps = make_in_maps(inputs)
    res = bass_utils.run_bass_kernel_spmd(
        nc, in_maps, core_ids=list(range(N_CORES)))
    return assemble_output([r["logits"] for r in res.results])
